# revision 1
# baseline (speedup 1.0000x reference)
"""Pairwise cosine-similarity adjacency (exp(-0.5 * cos_sim)) on 8 trn2 cores.

Input : x [4, 4096, 512] fp32
Output: exp(-0.5 * (xn @ xn.T)) per batch -> [4, 4096, 4096] fp32,
        xn = x / max(||x||_row, 1e-8)

Sharding (symmetry-aware): batch b = core // 2. The 4096x4096 adjacency is
symmetric, so in units of 1024x1024 quarter-blocks Q[i][j] (i,j in 0..3) only
a triangle cover is computed on-device; the host mirrors the rest.

  core even (own rows 0..2047 of batch b) computes
      dtop = rows 0..1023    x cols 0..2047    (Q00, Q01)
      dbot = rows 1024..2047 x cols 1024..2047 (Q11)
      outc = [rows 0..1023    x cols 2048..3071 (Q02);
              rows 1024..2047 x cols 3072..4095 (Q13)]
  core odd runs the same SPMD program fed own = rows 2048..4095 and
      cross = rows [1024..2047, 0..1023], producing Q22/Q23/Q33 and
      Q21, Q30.
  Host mirrors Q01.T, Q02.T, Q13.T, Q21.T, Q30.T into the lower copies.

Per-core pipeline:
  phase 1: 32 row tiles [128,512] (16 own + 16 cross): DMA in, ACT
           Square+accum into packed [128,8] group tiles, batched
           reciprocal+Sqrt -> inv, DVE normalize (cast f32r), PE
           transpose (f32r) into xnT tiles [128, 2048].
  phase 2: 320 f32r matmuls (K=128,M=128,N=512) into [128,1024] PSUM;
           ACT Exp(scale=-0.5) -> SBUF; DMA out.
"""
import sys

sys.path.insert(0, '/opt/trn_rl_repo')

import numpy as np

B, N, D = 4, 4096, 512
N_CORES = 8
R = N // 2      # 2048 own rows per core
Q = N // 4      # 1024 quarter-block size
EPS = 1e-8

_compiled = {}


def _build():
    import concourse.mybir as mybir
    import concourse.tile as tile
    from concourse import bacc
    from concourse.masks import make_identity

    fp32 = mybir.dt.float32
    f32r = mybir.dt.float32r

    nc = bacc.Bacc(trn_type="TRN2", target_bir_lowering=False, debug=False,
                   num_devices=N_CORES)
    xown = nc.dram_tensor("xown", [R, D], fp32, kind="ExternalInput")
    xcross = nc.dram_tensor("xcross", [R, D], fp32, kind="ExternalInput")
    dtop = nc.dram_tensor("dtop", [Q, 2 * Q], fp32, kind="ExternalOutput")
    dbot = nc.dram_tensor("dbot", [Q, Q], fp32, kind="ExternalOutput")
    outc = nc.dram_tensor("outc", [2 * Q, Q], fp32, kind="ExternalOutput")

    K_TILES = D // 128   # 4 contraction chunks
    NW = 1024            # psum accumulate width (2 banks)

    with tile.TileContext(nc) as tc:
        with tc.tile_pool(name="consts", bufs=1) as consts, \
             tc.tile_pool(name="xn_store", bufs=1) as xn_store, \
             tc.tile_pool(name="p1", bufs=6) as p1, \
             tc.tile_pool(name="p1psum", bufs=2, space="PSUM") as p1psum, \
             tc.tile_pool(name="p2psum", bufs=3, space="PSUM") as p2psum, \
             tc.tile_pool(name="p2out", bufs=4) as p2out:

            identf = consts.tile([128, 128], fp32)
            make_identity(nc, identf[:])
            ident = consts.tile([128, 128], f32r)
            nc.vector.tensor_copy(ident[:], identf[:])

            # xnT[k][s]: s=0 own rows transposed, s=1 cross rows transposed
            xnT = [[xn_store.tile([128, 2 * Q], f32r, name=f"xnT_{k}_{s}")
                    for s in range(2)] for k in range(K_TILES)]
            # packed norms^2, groups of 8 row tiles
            sqh = [xn_store.tile([128, 8], fp32, name=f"sqh_{g}")
                   for g in range(4)]
            invh = [xn_store.tile([128, 8], fp32, name=f"invh_{g}")
                    for g in range(4)]

            srcs = [xown, xcross]
            xts = {}

            def phase1_load(r):          # r in 0..31; side s = r // 16
                s, g, j = r // 16, r // 8, r % 8
                row0 = (r % 16) * 128
                xt = p1.tile([128, D], fp32, tag="xt", bufs=18, name=f"xt_{r}")
                nc.sync.dma_start(xt[:], srcs[s].ap()[row0:row0 + 128, :])
                xts[r] = xt
                scratch = p1.tile([128, D], fp32, tag="scratch", bufs=2)
                nc.scalar.activation(scratch[:], xt[:],
                                     mybir.ActivationFunctionType.Square,
                                     accum_out=sqh[g][:, j:j + 1])

            def phase1_inv(g):
                nc.vector.tensor_scalar_max(sqh[g][:], sqh[g][:], EPS * EPS)
                nc.vector.reciprocal(invh[g][:], sqh[g][:])
                nc.scalar.activation(invh[g][:], invh[g][:],
                                     mybir.ActivationFunctionType.Sqrt)

            def phase1_tp(r):
                s, g, j = r // 16, r // 8, r % 8
                c0 = (r % 16) * 128
                xt = xts[r]
                xnrm = p1.tile([128, D], f32r, tag="xnrm")
                nc.vector.tensor_scalar_mul(xnrm[:], xt[:], invh[g][:, j:j + 1])
                for k in range(K_TILES):
                    pt = p1psum.tile([128, 128], f32r, tag="tp")
                    nc.tensor.transpose(pt[:], xnrm[:, k * 128:(k + 1) * 128],
                                        ident[:])
                    nc.vector.tensor_copy(xnT[k][s][:, c0:c0 + 128], pt[:])

            def gemm(m, s, col0, dst, drow0, dcol0):
                """One [128, NW] output tile: own row block m, rhs side s,
                rhs cols col0.., DMA'd to dst[drow0.., dcol0..]."""
                acc = p2psum.tile([128, NW], fp32, tag="acc")
                for nn in range(NW // 512):
                    c = col0 + nn * 512
                    for k in range(K_TILES):
                        nc.tensor.matmul(
                            acc[:, nn * 512:(nn + 1) * 512],
                            xnT[k][0][:, m * 128:(m + 1) * 128],
                            xnT[k][s][:, c:c + 512],
                            start=(k == 0), stop=(k == K_TILES - 1))
                ot = p2out.tile([128, NW], fp32, tag="ot")
                nc.scalar.activation(ot[:], acc[:],
                                     mybir.ActivationFunctionType.Exp,
                                     scale=-0.5)
                nc.sync.dma_start(
                    dst.ap()[drow0:drow0 + 128, dcol0:dcol0 + NW], ot[:])

            for g in range(4):
                for r in range(g * 8, g * 8 + 8):
                    phase1_load(r)
                phase1_inv(g)
                for r in range(g * 8, g * 8 + 8):
                    phase1_tp(r)

            for m in range(8):                      # Q00, Q01
                for gcol in range(2):
                    gemm(m, 0, gcol * Q, dtop, m * 128, gcol * Q)
            for m in range(8, 16):                  # Q11
                gemm(m, 0, Q, dbot, (m - 8) * 128, 0)
            for m in range(16):                     # Q02 / Q13 (cross)
                gemm(m, 1, (m // 8) * Q, outc, m * 128, 0)

    nc.compile()
    return nc


def _in_maps(x):
    maps = []
    for c in range(N_CORES):
        b = c // 2
        xb = x[b]
        if c % 2 == 0:
            maps.append({"xown": xb[0:R],
                         "xcross": np.ascontiguousarray(xb[R:N])})
        else:
            maps.append({"xown": np.ascontiguousarray(xb[R:N]),
                         "xcross": np.concatenate([xb[Q:2 * Q], xb[0:Q]])})
    return maps


def _assemble(results, out):
    for c in range(N_CORES):
        b, odd = c // 2, c % 2
        o = out[b]
        r0 = odd * 2 * Q                  # own-row offset: 0 or 2048
        dtop = results[c]["dtop"]
        dbot = results[c]["dbot"]
        outc = results[c]["outc"]
        o[r0:r0 + Q, r0:r0 + 2 * Q] = dtop
        o[r0 + Q:r0 + 2 * Q, r0 + Q:r0 + 2 * Q] = dbot
        o[r0 + Q:r0 + 2 * Q, r0:r0 + Q] = dtop[:, Q:2 * Q].T
        # cross cols: even core -> [2048.., 3072..]; odd -> [1024.., 0..]
        ccol = [2 * Q, 3 * Q] if not odd else [Q, 0]
        for half in range(2):
            blk = outc[half * Q:(half + 1) * Q]
            rr = r0 + half * Q
            cc = ccol[half]
            o[rr:rr + Q, cc:cc + Q] = blk
            o[cc:cc + Q, rr:rr + Q] = blk.T
    return out


def kernel(x: np.ndarray) -> np.ndarray:
    from concourse.bass_utils import run_bass_kernel_spmd

    x = np.asarray(x, dtype=np.float32)
    assert x.shape == (B, N, D)

    if "nc" not in _compiled:
        _compiled["nc"] = _build()
    nc = _compiled["nc"]

    res = run_bass_kernel_spmd(nc, _in_maps(x), list(range(N_CORES)))
    out = np.empty((B, N, N), dtype=np.float32)
    return _assemble([res.results[c] for c in range(N_CORES)], out)



# revision 3
# speedup vs baseline: 1.0873x; 1.0873x over previous
"""Pairwise cosine-similarity adjacency (exp(-0.5 * cos_sim)) on 8 trn2 cores.

Input : x [4, 4096, 512] fp32
Output: exp(-0.5 * (xn @ xn.T)) per batch -> [4, 4096, 4096] fp32,
        xn = x / max(||x||_row, 1e-8)

Sharding (symmetry-aware): batch b = core // 2. The 4096x4096 adjacency is
symmetric, so in units of 1024x1024 quarter-blocks Q[i][j] (i,j in 0..3) only
a triangle cover is computed on-device; the host mirrors the rest.

  core even (own rows 0..2047 of batch b) computes
      dtop = rows 0..1023    x cols 0..2047    (Q00, Q01)
      dbot = rows 1024..2047 x cols 1024..2047 (Q11)
      outc = [rows 0..1023    x cols 2048..3071 (Q02);
              rows 1024..2047 x cols 3072..4095 (Q13)]
  core odd runs the same SPMD program fed own = rows 2048..4095 and
      cross = rows [1024..2047, 0..1023], producing Q22/Q23/Q33 and
      Q21, Q30.
  Host mirrors Q01.T, Q02.T, Q13.T, Q21.T, Q30.T into the lower copies.

Device pipeline (v2 — fp8 DoubleRow):
  Inputs arrive as bf16, host-permuted to [128, 16, 512] (partition-major).
  phase 1: per 4-row-tile batch: one DMA; ACT Square+accum -> norms^2;
           DVE reciprocal+Sqrt -> inv; normalize FUSED into the PE
           transpose as matmul against diag(inv) (bf16); PSUM fp32
           [c,t,r]-batched; copy-cast to fp8e4 DoubleRow pair tiles
           [128, 2, 2048] (DVE pair 0, GpSimd pair 1).
  phase 2: 160 fp8 DoubleRow matmuls (K=256/instr) into [128, 2048] PSUM
           fills; ACT Exp(scale=-0.5) -> bf16; DMA out. Host upcasts,
           assembles, mirrors.
"""
import sys

sys.path.insert(0, '/opt/trn_rl_repo')

import numpy as np
import ml_dtypes

B, N, D = 4, 4096, 512
N_CORES = 8
R = N // 2      # 2048 own rows per core
Q = N // 4      # 1024 quarter-block size
NT = R // 128   # 16 row tiles per side
EPS = 1e-8

_compiled = {}


def _build():
    import concourse.mybir as mybir
    import concourse.tile as tile
    from concourse import bacc
    from concourse.masks import make_identity

    fp32 = mybir.dt.float32
    bf16 = mybir.dt.bfloat16
    fp8 = mybir.dt.float8e4
    DR = mybir.MatmulPerfMode.DoubleRow

    nc = bacc.Bacc(trn_type="TRN2", target_bir_lowering=False, debug=False,
                   num_devices=N_CORES)
    xown = nc.dram_tensor("xown", [128, NT, D], bf16, kind="ExternalInput")
    xcross = nc.dram_tensor("xcross", [128, NT, D], bf16, kind="ExternalInput")
    dtop = nc.dram_tensor("dtop", [Q, 2 * Q], bf16, kind="ExternalOutput")
    dbot = nc.dram_tensor("dbot", [Q, Q], bf16, kind="ExternalOutput")
    outc = nc.dram_tensor("outc", [2 * Q, Q], bf16, kind="ExternalOutput")

    srcs = [xown, xcross]

    with tile.TileContext(nc) as tc:
        with tc.tile_pool(name="consts", bufs=1) as consts, \
             tc.tile_pool(name="store", bufs=1) as store, \
             tc.tile_pool(name="p1", bufs=3) as p1, \
             tc.tile_pool(name="psum", bufs=2, space="PSUM") as psum_pool, \
             tc.tile_pool(name="p2out", bufs=3) as p2out:

            identf = consts.tile([128, 128], fp32)
            make_identity(nc, identf[:])
            identb = consts.tile([128, 128], bf16)
            nc.vector.tensor_copy(identb[:], identf[:])

            # fp8 DoubleRow pair tiles: xnT[s][P][p, i, col] = xn[col, kd]
            # with kd = 256*P + 128*i + p   (s: 0 own rows, 1 cross rows)
            xnT = [[store.tile([128, 2, R], fp8, name=f"xnT_{s}_{P}")
                    for P in range(2)] for s in range(2)]
            sqh = [store.tile([128, 8], fp32, name=f"sqh_{g}")
                   for g in range(4)]
            invh = [store.tile([128, 8], fp32, name=f"invh_{g}")
                    for g in range(4)]

            def p1_load(b):
                """Batch b (0..7): 4 row tiles. Returns xb tile."""
                s, lt0 = b // 4, (b % 4) * 4
                g = b // 2
                xb = p1.tile([128, 4, D], bf16, tag="xb", name=f"xb_{b}")
                nc.sync.dma_start(xb[:], srcs[s].ap()[:, lt0:lt0 + 4, :])
                for t in range(4):
                    j = (b % 2) * 4 + t
                    scr = p1.tile([128, D], bf16, tag="scr", bufs=2)
                    nc.scalar.activation(scr[:], xb[:, t, :],
                                         mybir.ActivationFunctionType.Square,
                                         accum_out=sqh[g][:, j:j + 1])
                return xb

            def p1_inv(g):
                nc.vector.tensor_scalar_max(sqh[g][:], sqh[g][:], EPS * EPS)
                nc.vector.reciprocal(invh[g][:], sqh[g][:])
                nc.scalar.activation(invh[g][:], invh[g][:],
                                     mybir.ActivationFunctionType.Sqrt)

            def p1_transpose(b, xb):
                """Normalize+transpose batch b into fp8 pair tiles."""
                s, lt0, g = b // 4, (b % 4) * 4, b // 2
                pt = psum_pool.tile([128, 4, 4, 128], fp32, tag="ps",
                                    name=f"pt_{b}")
                for t in range(4):
                    j = (b % 2) * 4 + t
                    diag = p1.tile([128, 128], bf16, tag="diag", bufs=4)
                    nc.vector.tensor_scalar_mul(diag[:], identb[:],
                                                invh[g][:, j:j + 1])
                    for c in range(4):
                        nc.tensor.matmul(pt[:, c, t, :],
                                         xb[:, t, c * 128:(c + 1) * 128],
                                         diag[:], start=True, stop=True)
                c0 = lt0 * 128
                nc.vector.tensor_copy(xnT[s][0][:, :, c0:c0 + 512],
                                      pt[:, 0:2, :, :])
                nc.vector.tensor_copy(xnT[s][1][:, :, c0:c0 + 512],
                                      pt[:, 2:4, :, :])

            for g in range(4):
                xbs = [p1_load(2 * g), p1_load(2 * g + 1)]
                p1_inv(g)
                p1_transpose(2 * g, xbs[0])
                p1_transpose(2 * g + 1, xbs[1])

            # phase 2: units of [128 rows x 1024 cols]; m = own row tile,
            # s/c0 = rhs side and local col; dst (tensor, row, col).
            units = []
            for m in range(8):
                units.append((m, 0, 0, dtop, m * 128, 0))
                units.append((m, 0, 1024, dtop, m * 128, 1024))
                units.append((m, 1, 0, outc, m * 128, 0))
            for m in range(8, 16):
                units.append((m, 0, 1024, dbot, (m - 8) * 128, 0))
                units.append((m, 1, 1024, outc, m * 128, 0))

            for f in range(0, len(units), 2):
                pair = units[f:f + 2]
                acc = psum_pool.tile([128, 2048], fp32, tag="ps",
                                     name=f"acc_{f}")
                for P in range(2):
                    for u, (m, s, c0, _, _, _) in enumerate(pair):
                        for h in range(2):
                            col = u * 1024 + h * 512
                            nc.tensor.matmul(
                                acc[:, col:col + 512],
                                xnT[0][P][:, :, m * 128:(m + 1) * 128],
                                xnT[s][P][:, :, c0 + h * 512:c0 + h * 512 + 512],
                                start=(P == 0), stop=(P == 1),
                                perf_mode=DR)
                ot = p2out.tile([128, 2048], bf16, tag="ot", name=f"ot_{f}")
                nc.scalar.activation(ot[:], acc[:],
                                     mybir.ActivationFunctionType.Exp,
                                     scale=-0.5)
                for u, (m, s, c0, dst, drow, dcol) in enumerate(pair):
                    nc.sync.dma_start(
                        dst.ap()[drow:drow + 128, dcol:dcol + 1024],
                        ot[:, u * 1024:(u + 1) * 1024])

    nc.compile()
    return nc


def _permute(rows_bf16):
    """[2048, 512] bf16 -> [128, 16, 512] partition-major contiguous."""
    return np.ascontiguousarray(
        rows_bf16.reshape(NT, 128, D).transpose(1, 0, 2))


def _in_maps(x):
    xb16 = x.astype(ml_dtypes.bfloat16)
    maps = []
    for c in range(N_CORES):
        b = c // 2
        xb = xb16[b]
        if c % 2 == 0:
            own, cross = xb[0:R], xb[R:N]
        else:
            own = xb[R:N]
            cross = np.concatenate([xb[Q:2 * Q], xb[0:Q]])
        maps.append({"xown": _permute(own), "xcross": _permute(cross)})
    return maps


def _assemble(results, out):
    for c in range(N_CORES):
        b, odd = c // 2, c % 2
        o = out[b]
        r0 = odd * 2 * Q                  # own-row offset: 0 or 2048
        dtop = np.asarray(results[c]["dtop"]).astype(np.float32)
        dbot = np.asarray(results[c]["dbot"]).astype(np.float32)
        outc = np.asarray(results[c]["outc"]).astype(np.float32)
        o[r0:r0 + Q, r0:r0 + 2 * Q] = dtop
        o[r0 + Q:r0 + 2 * Q, r0 + Q:r0 + 2 * Q] = dbot
        o[r0 + Q:r0 + 2 * Q, r0:r0 + Q] = dtop[:, Q:2 * Q].T
        # cross cols: even core -> [2048.., 3072..]; odd -> [1024.., 0..]
        ccol = [2 * Q, 3 * Q] if not odd else [Q, 0]
        for half in range(2):
            blk = outc[half * Q:(half + 1) * Q]
            rr = r0 + half * Q
            cc = ccol[half]
            o[rr:rr + Q, cc:cc + Q] = blk
            o[cc:cc + Q, rr:rr + Q] = blk.T
    return out


def kernel(x: np.ndarray) -> np.ndarray:
    from concourse.bass_utils import run_bass_kernel_spmd

    x = np.asarray(x, dtype=np.float32)
    assert x.shape == (B, N, D)

    if "nc" not in _compiled:
        _compiled["nc"] = _build()
    nc = _compiled["nc"]

    res = run_bass_kernel_spmd(nc, _in_maps(x), list(range(N_CORES)))
    out = np.empty((B, N, N), dtype=np.float32)
    return _assemble([res.results[c] for c in range(N_CORES)], out)
